# revision 15
# baseline (speedup 1.0000x reference)
"""Trainium2 Bass kernel for CFContrastiveLoss.

Reference semantics (per sample of N=16 options, D=768 dims):
  - L2-normalize option embeddings
  - sim = pairwise cosine sims within the sample (16x16 gram)
  - max_neg[n] = max over negative-labeled columns of sim[n, :]
  - loss = mean over (positive rows of valid samples) of relu(max_neg + 0.3)

Device strategy (pure data parallel over batch, 8 cores):
  - 128 rows (= 8 samples x 16 options) per "group"; per core 16384 rows
    = 128 groups, DMA'd in super-groups of 8 (1.57 MB per transfer, hi and
    lo on separate HWDGE rings) for near-peak HBM bandwidth.
  - Host pre-normalizes embeddings, splits each fp32 value into a bf16
    (hi, lo) pair (hi = bf16(x), lo = bf16(x - hi)) and pre-transposes to
    the matmul layout.  The gram matrix is computed on the TensorEngine as
    Hi.T@Hi + Hi.T@Lo + Lo.T@Hi (the Lo.T@Lo term is ~1e-7 and dropped),
    giving fp32-grade accuracy at bf16 matmul speed (fp32 matmuls are 4x
    slower on TRN2).  DMA volume is identical to fp32 (2x2B per element).
  - The label/validity masking is folded into the same PSUM accumulation
    as one extra K=10 matmul of +-2^30 sentinel outer products:
      row 0:  ones x negc           (negc[m] = -2^30 iff label[m] == 1)
      row 1:  ones x (-2^30 * ones) (mask everything ...)
      row 2+s: u_s x (+2^30 * u_s)  (... except within-sample blocks)
    Sentinels are powers of two, so in-block negative columns get an
    exactly-zero mask contribution and unmasked sims are bit-exact.
  - Per group the device then does a single VectorE row-max from PSUM.
    relu/margin/weighting/final mean are O(rows) and done on host.
"""

import os

import numpy as np
import ml_dtypes

import concourse.bass as bass
import concourse.mybir as mybir
from concourse import bacc, tile
from concourse.bass_utils import run_bass_kernel_spmd

BF16 = mybir.dt.bfloat16
F32 = mybir.dt.float32

B, N, D = 8192, 16, 768
N_CORES = 8
ROWS = B * N                      # 131072
ROWS_PER_CORE = ROWS // N_CORES   # 16384
GROUPS = ROWS_PER_CORE // 128     # 128 groups of 128 rows per core
KCH = D // 128                    # 6 contraction chunks
SG = 8                            # groups per super-group (one DMA batch)
N_SG = GROUPS // SG               # 16
MASK_K = 2 + 128 // N             # 10 mask matmul rows
SENT = np.float32(2.0 ** 30)
MARGIN = np.float32(0.3)

_CACHE: dict = {}

LAST_RESULT = None  # BassKernelResults of the most recent device run


def _build_program() -> bass.Bass:
    nc = bacc.Bacc(None)
    ethi = nc.declare_dram_parameter("ethi", [N_SG, 128, SG * D], BF16, isOutput=False)
    etlo = nc.declare_dram_parameter("etlo", [N_SG, 128, SG * D], BF16, isOutput=False)
    mrhs = nc.declare_dram_parameter("mrhs", [MASK_K, GROUPS * 128], BF16, isOutput=False)
    mlhs = nc.declare_dram_parameter("mlhs", [128, 128], BF16, isOutput=False)
    out = nc.declare_dram_parameter("out", [N_SG, 128, SG], F32, isOutput=True)

    with tile.TileContext(nc) as tc:
        with (
            tc.tile_pool(name="emb", bufs=3) as emb_pool,
            tc.tile_pool(name="const", bufs=1) as const_pool,
            tc.tile_pool(name="wide", bufs=2) as wide_pool,
            tc.tile_pool(name="psum", bufs=8, space="PSUM") as psum_pool,
        ):
            # Mask operands padded to K=128 so every matmul runs with the
            # same full-array tile config (a K=10 matmul between K=128 ones
            # costs ~2x95ns of PE reconfig per group).  lhsT rows MASK_K..127
            # are zero, so the uninitialized rhs pad rows contribute 0 -- but
            # memset them anyway so no NaN*0 can leak in.
            mlhs_t = const_pool.tile([128, 128], BF16)
            nc.sync.dma_start(mlhs_t[:], mlhs[:])
            mr_all = const_pool.tile([128, GROUPS * 128], BF16)
            nc.vector.memset(mr_all[:, :], 0.0)
            nc.scalar.dma_start(mr_all[:MASK_K, :], mrhs[:])

            for sg in range(N_SG):
                hi = emb_pool.tile([128, SG * D], BF16, tag="hi")
                lo = emb_pool.tile([128, SG * D], BF16, tag="lo")
                nc.sync.dma_start(hi[:], ethi[sg])
                nc.scalar.dma_start(lo[:], etlo[sg])
                wide = wide_pool.tile([128, SG], F32)
                for gi in range(SG):
                    g = sg * SG + gi
                    ps = psum_pool.tile([128, 512], F32)  # one full PSUM bank
                    G = ps[:, 0:128]
                    # Mask sentinels first (start=True clears the bank).
                    nc.tensor.matmul(
                        G, mlhs_t[:], mr_all[:, g * 128:(g + 1) * 128],
                        start=True, stop=False,
                    )
                    for k in range(KCH):
                        c0 = (gi * KCH + k) * 128
                        hk = hi[:, c0:c0 + 128]
                        lk = lo[:, c0:c0 + 128]
                        nc.tensor.matmul(G, hk, hk, start=False, stop=False)
                        nc.tensor.matmul(G, hk, lk, start=False, stop=False)
                        nc.tensor.matmul(G, lk, hk, start=False, stop=(k == KCH - 1))
                    nc.vector.reduce_max(wide[:, gi:gi + 1], G, axis=mybir.AxisListType.X)
                nc.sync.dma_start(out[sg], wide[:])
    nc.finalize()
    return nc


def _prep_core_inputs(Xn: np.ndarray, lab: np.ndarray):
    """Per-core input maps from normalized embeddings + flat labels."""
    hi = Xn.astype(ml_dtypes.bfloat16)
    lo = (Xn - hi.astype(np.float32)).astype(ml_dtypes.bfloat16)

    negc = np.where(lab == 1, -SENT, np.float32(0.0)).astype(np.float32)

    m_idx = np.arange(128)
    # mask lhsT: [128, 128], rows = [ones; ones; u_0..u_7; zeros...]
    mlhs = np.zeros((128, 128), dtype=np.float32)
    mlhs[0, :] = 1.0
    mlhs[1, :] = 1.0
    for s in range(128 // N):
        mlhs[2 + s, :] = (m_idx // N == s).astype(np.float32)
    mlhs_bf = mlhs.astype(ml_dtypes.bfloat16)

    # static part of mask rhs rows 1..9 (per 128-column group)
    mrhs_static = np.zeros((MASK_K, 128), dtype=np.float32)
    mrhs_static[1, :] = -SENT
    for s in range(128 // N):
        mrhs_static[2 + s, :] = np.where(m_idx // N == s, SENT, np.float32(0.0))

    def to_sg_layout(a_core: np.ndarray) -> np.ndarray:
        # [rows=16384, 768] -> [sg, gi, n, k, p] -> [sg, p, gi, k, n]
        return np.ascontiguousarray(
            a_core.reshape(N_SG, SG, 128, KCH, 128).transpose(0, 4, 1, 3, 2)
        ).reshape(N_SG, 128, SG * D)

    in_maps = []
    for c in range(N_CORES):
        r0 = c * ROWS_PER_CORE
        r1 = r0 + ROWS_PER_CORE
        # mask rhs for all groups: [10, 16384]; row 0 = negc, rows 1.. static
        mr = np.empty((MASK_K, ROWS_PER_CORE), dtype=np.float32)
        mr[0, :] = negc[r0:r1]
        mr[1:, :] = np.tile(mrhs_static[1:, :], (1, GROUPS))
        in_maps.append({
            "ethi": to_sg_layout(hi[r0:r1]),
            "etlo": to_sg_layout(lo[r0:r1]),
            "mrhs": mr.astype(ml_dtypes.bfloat16),
            "mlhs": mlhs_bf,
        })
    return in_maps


def kernel(embeddings: np.ndarray, labels: np.ndarray) -> np.ndarray:
    global LAST_RESULT
    assert embeddings.shape == (B, N, D)
    assert labels.shape == (B, N)

    X = np.asarray(embeddings, dtype=np.float32).reshape(ROWS, D)
    lab = np.asarray(labels).reshape(ROWS)

    ss = np.square(X).sum(axis=1, dtype=np.float32)
    norms = np.sqrt(ss)
    Xn = X / np.maximum(norms, np.float32(1e-12))[:, None]

    in_maps = _prep_core_inputs(Xn, lab)

    if "nc" not in _CACHE:
        _CACHE["nc"] = _build_program()
    nc = _CACHE["nc"]

    trace = os.environ.get("BASS_KERNEL_TRACE", "0") == "1"
    res = run_bass_kernel_spmd(nc, in_maps, list(range(N_CORES)), trace=trace)
    LAST_RESULT = res

    # out[sg, p, gi]: group g = sg*SG+gi, row-within-group p
    maxneg = np.concatenate(
        [np.asarray(r["out"]).transpose(0, 2, 1).reshape(ROWS_PER_CORE)
         for r in res.results]
    )

    triplet = np.maximum(maxneg + MARGIN, np.float32(0.0))
    has_neg = (np.asarray(labels) == 0).any(axis=1)
    w = (lab == 1) & np.repeat(has_neg, N)
    loss_sum = np.float32((triplet * w).sum(dtype=np.float64))
    count = int(w.sum())
    loss = np.float32(loss_sum / np.float32(max(count, 1)))
    return np.asarray(loss, dtype=np.float32)


# revision 18
# speedup vs baseline: 1.0774x; 1.0774x over previous
"""Trainium2 Bass kernel for CFContrastiveLoss.

Reference semantics (per sample of N=16 options, D=768 dims):
  - L2-normalize option embeddings
  - sim = pairwise cosine sims within the sample (16x16 gram)
  - max_neg[n] = max over negative-labeled columns of sim[n, :]
  - loss = mean over (positive rows of valid samples) of relu(max_neg + 0.3)

Device strategy (pure data parallel over batch, 8 cores):
  - 128 rows (= 8 samples x 16 options) per "group"; per core 16384 rows
    = 128 groups, DMA'd in super-groups of 8 (1.57 MB per transfer, hi and
    lo on separate HWDGE rings) for near-peak HBM bandwidth.
  - Host pre-normalizes embeddings, splits each fp32 value into a bf16
    (hi, lo) pair (hi = bf16(x), lo = bf16(x - hi)) and pre-transposes to
    the matmul layout.  The gram matrix is computed on the TensorEngine as
    Hi.T@Hi + Hi.T@Lo + Lo.T@Hi (the Lo.T@Lo term is ~1e-7 and dropped),
    giving fp32-grade accuracy at bf16 matmul speed (fp32 matmuls are 4x
    slower on TRN2).  DMA volume is identical to fp32 (2x2B per element).
  - The label/validity masking is folded into the same PSUM accumulation
    as one extra K=10 matmul of +-2^30 sentinel outer products:
      row 0:  ones x negc           (negc[m] = -2^30 iff label[m] == 1)
      row 1:  ones x (-2^30 * ones) (mask everything ...)
      row 2+s: u_s x (+2^30 * u_s)  (... except within-sample blocks)
    Sentinels are powers of two, so in-block negative columns get an
    exactly-zero mask contribution and unmasked sims are bit-exact.
  - Per group the device then does a single VectorE row-max from PSUM.
    relu/margin/weighting/final mean are O(rows) and done on host.
"""

import os

import numpy as np
import ml_dtypes

import concourse.bass as bass
import concourse.mybir as mybir
from concourse import bacc, tile
from concourse.bass_utils import run_bass_kernel_spmd

BF16 = mybir.dt.bfloat16
F32 = mybir.dt.float32

B, N, D = 8192, 16, 768
N_CORES = 8
ROWS = B * N                      # 131072
ROWS_PER_CORE = ROWS // N_CORES   # 16384
GROUPS = ROWS_PER_CORE // 128     # 128 groups of 128 rows per core
KCH = D // 128                    # 6 contraction chunks
SG = 8                            # groups per super-group (one DMA batch)
N_SG = GROUPS // SG               # 16
MASK_K = 2 + 128 // N             # 10 mask matmul rows
SENT = np.float32(2.0 ** 30)
MARGIN = np.float32(0.3)

_CACHE: dict = {}

LAST_RESULT = None  # BassKernelResults of the most recent device run


def _build_program() -> bass.Bass:
    nc = bacc.Bacc(None)
    ethi = nc.declare_dram_parameter("ethi", [N_SG, 128, SG * D], BF16, isOutput=False)
    etlo = nc.declare_dram_parameter("etlo", [N_SG, 128, SG * D], BF16, isOutput=False)
    mrhs = nc.declare_dram_parameter("mrhs", [N_SG, MASK_K, SG * 128], BF16, isOutput=False)
    mlhs = nc.declare_dram_parameter("mlhs", [128, 128], BF16, isOutput=False)
    out = nc.declare_dram_parameter("out", [N_SG, 128, SG], F32, isOutput=True)

    with tile.TileContext(nc) as tc:
        with (
            tc.tile_pool(name="emb", bufs=3) as emb_pool,
            tc.tile_pool(name="const", bufs=1) as const_pool,
            tc.tile_pool(name="wide", bufs=2) as wide_pool,
            tc.tile_pool(name="psum", bufs=8, space="PSUM") as psum_pool,
        ):
            # Mask operands padded to K=128 so every matmul runs with the
            # same full-array tile config (a K=10 matmul between K=128 ones
            # costs PE reconfig stalls).  lhsT rows MASK_K..127 are zero, so
            # the rhs pad rows contribute 0 -- memset them once so no NaN*0
            # can leak in.  Two ping-pong mask tiles (one per super-group in
            # flight) keep the one-time memset off the critical path.
            mlhs_t = const_pool.tile([128, 128], BF16)
            nc.sync.dma_start(mlhs_t[:], mlhs[:])
            mr_tiles = []
            for i in range(2):
                mr_t = const_pool.tile([128, SG * 128], BF16, name=f"mr{i}")
                nc.vector.memset(mr_t[:, :], 0.0)
                mr_tiles.append(mr_t)

            for sg in range(N_SG):
                hi = emb_pool.tile([128, SG * D], BF16, tag="hi")
                lo = emb_pool.tile([128, SG * D], BF16, tag="lo")
                nc.sync.dma_start(hi[:], ethi[sg])
                nc.scalar.dma_start(lo[:], etlo[sg])
                mr_t = mr_tiles[sg % 2]
                nc.sync.dma_start(mr_t[:MASK_K, :], mrhs[sg])
                wide = wide_pool.tile([128, SG], F32)
                for gi in range(SG):
                    g = sg * SG + gi
                    ps = psum_pool.tile([128, 512], F32)  # one full PSUM bank
                    G = ps[:, 0:128]
                    # Mask sentinels first (start=True clears the bank).
                    nc.tensor.matmul(
                        G, mlhs_t[:], mr_t[:, gi * 128:(gi + 1) * 128],
                        start=True, stop=False,
                    )
                    for k in range(KCH):
                        c0 = (gi * KCH + k) * 128
                        hk = hi[:, c0:c0 + 128]
                        lk = lo[:, c0:c0 + 128]
                        nc.tensor.matmul(G, hk, hk, start=False, stop=False)
                        nc.tensor.matmul(G, hk, lk, start=False, stop=False)
                        nc.tensor.matmul(G, lk, hk, start=False, stop=(k == KCH - 1))
                    nc.vector.reduce_max(wide[:, gi:gi + 1], G, axis=mybir.AxisListType.X)
                nc.sync.dma_start(out[sg], wide[:])
    nc.finalize()
    return nc


def _prep_core_inputs(Xn: np.ndarray, lab: np.ndarray):
    """Per-core input maps from normalized embeddings + flat labels."""
    hi = Xn.astype(ml_dtypes.bfloat16)
    lo = (Xn - hi.astype(np.float32)).astype(ml_dtypes.bfloat16)

    negc = np.where(lab == 1, -SENT, np.float32(0.0)).astype(np.float32)

    m_idx = np.arange(128)
    # mask lhsT: [128, 128], rows = [ones; ones; u_0..u_7; zeros...]
    mlhs = np.zeros((128, 128), dtype=np.float32)
    mlhs[0, :] = 1.0
    mlhs[1, :] = 1.0
    for s in range(128 // N):
        mlhs[2 + s, :] = (m_idx // N == s).astype(np.float32)
    mlhs_bf = mlhs.astype(ml_dtypes.bfloat16)

    # static part of mask rhs rows 1..9 (per 128-column group)
    mrhs_static = np.zeros((MASK_K, 128), dtype=np.float32)
    mrhs_static[1, :] = -SENT
    for s in range(128 // N):
        mrhs_static[2 + s, :] = np.where(m_idx // N == s, SENT, np.float32(0.0))

    def to_sg_layout(a_core: np.ndarray) -> np.ndarray:
        # [rows=16384, 768] -> [sg, gi, n, k, p] -> [sg, p, gi, k, n]
        return np.ascontiguousarray(
            a_core.reshape(N_SG, SG, 128, KCH, 128).transpose(0, 4, 1, 3, 2)
        ).reshape(N_SG, 128, SG * D)

    in_maps = []
    for c in range(N_CORES):
        r0 = c * ROWS_PER_CORE
        r1 = r0 + ROWS_PER_CORE
        # mask rhs per super-group: [N_SG, 10, SG*128]; row 0 = negc,
        # rows 1.. static
        mr = np.empty((MASK_K, ROWS_PER_CORE), dtype=np.float32)
        mr[0, :] = negc[r0:r1]
        mr[1:, :] = np.tile(mrhs_static[1:, :], (1, GROUPS))
        mr = mr.reshape(MASK_K, N_SG, SG * 128).transpose(1, 0, 2)
        in_maps.append({
            "ethi": to_sg_layout(hi[r0:r1]),
            "etlo": to_sg_layout(lo[r0:r1]),
            "mrhs": np.ascontiguousarray(mr).astype(ml_dtypes.bfloat16),
            "mlhs": mlhs_bf,
        })
    return in_maps


def kernel(embeddings: np.ndarray, labels: np.ndarray) -> np.ndarray:
    global LAST_RESULT
    assert embeddings.shape == (B, N, D)
    assert labels.shape == (B, N)

    X = np.asarray(embeddings, dtype=np.float32).reshape(ROWS, D)
    lab = np.asarray(labels).reshape(ROWS)

    ss = np.square(X).sum(axis=1, dtype=np.float32)
    norms = np.sqrt(ss)
    Xn = X / np.maximum(norms, np.float32(1e-12))[:, None]

    in_maps = _prep_core_inputs(Xn, lab)

    if "nc" not in _CACHE:
        _CACHE["nc"] = _build_program()
    nc = _CACHE["nc"]

    trace = os.environ.get("BASS_KERNEL_TRACE", "0") == "1"
    res = run_bass_kernel_spmd(nc, in_maps, list(range(N_CORES)), trace=trace)
    LAST_RESULT = res

    # out[sg, p, gi]: group g = sg*SG+gi, row-within-group p
    maxneg = np.concatenate(
        [np.asarray(r["out"]).transpose(0, 2, 1).reshape(ROWS_PER_CORE)
         for r in res.results]
    )

    triplet = np.maximum(maxneg + MARGIN, np.float32(0.0))
    has_neg = (np.asarray(labels) == 0).any(axis=1)
    w = (lab == 1) & np.repeat(has_neg, N)
    loss_sum = np.float32((triplet * w).sum(dtype=np.float64))
    count = int(w.sum())
    loss = np.float32(loss_sum / np.float32(max(count, 1)))
    return np.asarray(loss, dtype=np.float32)


# revision 19
# speedup vs baseline: 1.5480x; 1.4368x over previous
"""Trainium2 Bass kernel for CFContrastiveLoss.

Reference semantics (per sample of N=16 options, D=768 dims):
  - L2-normalize option embeddings
  - sim = pairwise cosine sims within the sample (16x16 gram)
  - max_neg[n] = max over negative-labeled columns of sim[n, :]
  - loss = mean over (positive rows of valid samples) of relu(max_neg + 0.3)

Device strategy (pure data parallel over batch, 8 cores):
  - 128 rows (= 8 samples x 16 options) per "group"; per core 16384 rows
    = 128 groups, DMA'd in super-groups of 8 (1.57 MB per transfer,
    alternating between the two HWDGE rings) for near-peak HBM bandwidth.
  - Host pre-normalizes embeddings, casts to fp16 and pre-transposes to
    the matmul layout.  The per-sample gram matrices are computed on the
    TensorEngine as block-diagonal 128x128 grams (fp32 PSUM accumulate).
    fp16 elements carry ~11 mantissa bits; the resulting per-sim error
    (~1e-5) averages out over the ~52k contributing rows, measured final
    loss error <= 2e-7 across seeds (fp32 matmuls would be 4x slower,
    bf16 hi+lo pairs need 2x the DMA bytes and 3x the matmuls).
  - The label/validity masking is folded into the same PSUM accumulation
    as one extra matmul of +-2^14 sentinel outer products (fp16-exact
    powers of two):
      row 0:  ones x negc           (negc[m] = -2^14 iff label[m] == 1)
      row 1:  ones x (-2^14 * ones) (mask everything ...)
      row 2+s: u_s x (+2^14 * u_s)  (... except within-sample blocks)
    Sentinels cancel exactly, so in-block negative columns get an
    exactly-zero mask contribution and unmasked sims are bit-exact;
    masked entries sit at <= -2^14 + 1 so relu(max + margin) = 0.
    Mask operands are zero-padded to K=128 so every matmul runs the same
    full-array tile config (mixed tile sizes cost PE reconfig stalls).
  - Per group the device then does a single VectorE row-max from PSUM.
    relu/margin/weighting/final mean are O(rows) and done on host.
"""

import os

import numpy as np

import concourse.bass as bass
import concourse.mybir as mybir
from concourse import bacc, tile
from concourse.bass_utils import run_bass_kernel_spmd

FP16 = mybir.dt.float16
F32 = mybir.dt.float32

B, N, D = 8192, 16, 768
N_CORES = 8
ROWS = B * N                      # 131072
ROWS_PER_CORE = ROWS // N_CORES   # 16384
GROUPS = ROWS_PER_CORE // 128     # 128 groups of 128 rows per core
KCH = D // 128                    # 6 contraction chunks
SG = 8                            # groups per super-group (one DMA batch)
N_SG = GROUPS // SG               # 16
MASK_K = 2 + 128 // N             # 10 live mask matmul rows
SENT = np.float32(2.0 ** 14)      # fp16-exact sentinel
MARGIN = np.float32(0.3)

_CACHE: dict = {}

LAST_RESULT = None  # BassKernelResults of the most recent device run


def _build_program() -> bass.Bass:
    nc = bacc.Bacc(None)
    et = nc.declare_dram_parameter("et", [N_SG, 128, SG * D], FP16, isOutput=False)
    mrhs = nc.declare_dram_parameter("mrhs", [N_SG, MASK_K, SG * 128], FP16, isOutput=False)
    mlhs = nc.declare_dram_parameter("mlhs", [128, 128], FP16, isOutput=False)
    out = nc.declare_dram_parameter("out", [N_SG, 128, SG], F32, isOutput=True)

    with tile.TileContext(nc) as tc:
        with (
            tc.tile_pool(name="emb", bufs=3) as emb_pool,
            tc.tile_pool(name="const", bufs=1) as const_pool,
            tc.tile_pool(name="wide", bufs=2) as wide_pool,
            tc.tile_pool(name="psum", bufs=8, space="PSUM") as psum_pool,
        ):
            mlhs_t = const_pool.tile([128, 128], FP16)
            nc.sync.dma_start(mlhs_t[:], mlhs[:])
            # Two ping-pong mask-rhs tiles; pad rows (MASK_K..127) x zero
            # lhsT rows contribute nothing -- memset once so no NaN*0.
            mr_tiles = []
            for i in range(2):
                mr_t = const_pool.tile([128, SG * 128], FP16, name=f"mr{i}")
                nc.vector.memset(mr_t[:, :], 0.0)
                mr_tiles.append(mr_t)

            for sg in range(N_SG):
                hi = emb_pool.tile([128, SG * D], FP16, tag="hi")
                # Alternate the embedding stream between the two HWDGE rings.
                dma_eng = nc.sync if sg % 2 == 0 else nc.scalar
                dma_eng.dma_start(hi[:], et[sg])
                mr_t = mr_tiles[sg % 2]
                nc.sync.dma_start(mr_t[:MASK_K, :], mrhs[sg])
                wide = wide_pool.tile([128, SG], F32)
                for gi in range(SG):
                    ps = psum_pool.tile([128, 512], F32)  # one full PSUM bank
                    G = ps[:, 0:128]
                    # Mask sentinels first (start=True clears the bank).
                    nc.tensor.matmul(
                        G, mlhs_t[:], mr_t[:, gi * 128:(gi + 1) * 128],
                        start=True, stop=False,
                    )
                    for k in range(KCH):
                        c0 = (gi * KCH + k) * 128
                        hk = hi[:, c0:c0 + 128]
                        nc.tensor.matmul(G, hk, hk, start=False, stop=(k == KCH - 1))
                    nc.vector.reduce_max(wide[:, gi:gi + 1], G, axis=mybir.AxisListType.X)
                nc.sync.dma_start(out[sg], wide[:])
    nc.finalize()
    return nc


def _prep_core_inputs(Xn16: np.ndarray, lab: np.ndarray):
    """Per-core input maps from fp16-normalized embeddings + flat labels."""
    negc = np.where(lab == 1, -SENT, np.float32(0.0)).astype(np.float16)

    m_idx = np.arange(128)
    # mask lhsT: [128, 128], rows = [ones; ones; u_0..u_7; zeros...]
    mlhs = np.zeros((128, 128), dtype=np.float16)
    mlhs[0, :] = 1.0
    mlhs[1, :] = 1.0
    for s in range(128 // N):
        mlhs[2 + s, :] = (m_idx // N == s).astype(np.float16)

    # static part of mask rhs rows 1..9 (per 128-column group)
    mrhs_static = np.zeros((MASK_K, 128), dtype=np.float16)
    mrhs_static[1, :] = -SENT
    for s in range(128 // N):
        mrhs_static[2 + s, :] = np.where(m_idx // N == s, SENT, 0.0).astype(np.float16)

    def to_sg_layout(a_core: np.ndarray) -> np.ndarray:
        # [rows=16384, 768] -> [sg, gi, n, k, p] -> [sg, p, gi, k, n]
        return np.ascontiguousarray(
            a_core.reshape(N_SG, SG, 128, KCH, 128).transpose(0, 4, 1, 3, 2)
        ).reshape(N_SG, 128, SG * D)

    in_maps = []
    for c in range(N_CORES):
        r0 = c * ROWS_PER_CORE
        r1 = r0 + ROWS_PER_CORE
        # mask rhs per super-group: [N_SG, 10, SG*128]
        mr = np.empty((MASK_K, ROWS_PER_CORE), dtype=np.float16)
        mr[0, :] = negc[r0:r1]
        mr[1:, :] = np.tile(mrhs_static[1:, :], (1, GROUPS))
        mr = mr.reshape(MASK_K, N_SG, SG * 128).transpose(1, 0, 2)
        in_maps.append({
            "et": to_sg_layout(Xn16[r0:r1]),
            "mrhs": np.ascontiguousarray(mr),
            "mlhs": mlhs,
        })
    return in_maps


def kernel(embeddings: np.ndarray, labels: np.ndarray) -> np.ndarray:
    global LAST_RESULT
    assert embeddings.shape == (B, N, D)
    assert labels.shape == (B, N)

    X = np.asarray(embeddings, dtype=np.float32).reshape(ROWS, D)
    lab = np.asarray(labels).reshape(ROWS)

    ss = np.square(X).sum(axis=1, dtype=np.float32)
    norms = np.sqrt(ss)
    Xn16 = (X / np.maximum(norms, np.float32(1e-12))[:, None]).astype(np.float16)

    in_maps = _prep_core_inputs(Xn16, lab)

    if "nc" not in _CACHE:
        _CACHE["nc"] = _build_program()
    nc = _CACHE["nc"]

    trace = os.environ.get("BASS_KERNEL_TRACE", "0") == "1"
    res = run_bass_kernel_spmd(nc, in_maps, list(range(N_CORES)), trace=trace)
    LAST_RESULT = res

    # out[sg, p, gi]: group g = sg*SG+gi, row-within-group p
    maxneg = np.concatenate(
        [np.asarray(r["out"]).transpose(0, 2, 1).reshape(ROWS_PER_CORE)
         for r in res.results]
    )

    triplet = np.maximum(maxneg + MARGIN, np.float32(0.0))
    has_neg = (np.asarray(labels) == 0).any(axis=1)
    w = (lab == 1) & np.repeat(has_neg, N)
    loss_sum = np.float32((triplet * w).sum(dtype=np.float64))
    count = int(w.sum())
    loss = np.float32(loss_sum / np.float32(max(count, 1)))
    return np.asarray(loss, dtype=np.float32)


# revision 21
# speedup vs baseline: 2.0750x; 1.3405x over previous
"""Trainium2 Bass kernel for CFContrastiveLoss.

Reference semantics (per sample of N=16 options, D=768 dims):
  - L2-normalize option embeddings
  - sim = pairwise cosine sims within the sample (16x16 gram)
  - max_neg[n] = max over negative-labeled columns of sim[n, :]
  - loss = mean over (positive rows of valid samples) of relu(max_neg + 0.3)

Device strategy (pure data parallel over batch, 8 cores):
  - 128 rows (= 8 samples x 16 options) per "group"; per core 16384 rows
    = 128 groups, DMA'd in super-groups of 8 (1.57 MB per transfer,
    alternating between the two HWDGE rings) for near-peak HBM bandwidth.
  - Host pre-normalizes embeddings, casts to fp16 and pre-transposes to
    the matmul layout.  The per-sample gram matrices are computed on the
    TensorEngine as block-diagonal 128x128 grams (fp32 PSUM accumulate).
    fp16 elements carry ~11 mantissa bits; the resulting per-sim error
    (~1e-5) averages out over the ~52k contributing rows, measured final
    loss error <= 2e-7 across seeds (fp32 matmuls would be 4x slower,
    bf16 hi+lo pairs need 2x the DMA bytes and 3x the matmuls).
  - The label/validity masking is folded into the same PSUM accumulation
    as one extra matmul of +-2^14 sentinel outer products (fp16-exact
    powers of two):
      row 0:  ones x negc           (negc[m] = -2^14 iff label[m] == 1)
      row 1:  ones x (-2^14 * ones) (mask everything ...)
      row 2+s: u_s x (+2^14 * u_s)  (... except within-sample blocks)
    Sentinels cancel exactly, so in-block negative columns get an
    exactly-zero mask contribution and unmasked sims are bit-exact;
    masked entries sit at <= -2^14 + 1 so relu(max + margin) = 0.
    Mask operands are zero-padded to K=128 so every matmul runs the same
    full-array tile config (mixed tile sizes cost PE reconfig stalls).
  - Per group the device then does a single VectorE row-max from PSUM.
    relu/margin/weighting/final mean are O(rows) and done on host.
"""

import os

import numpy as np

import concourse.bass as bass
import concourse.mybir as mybir
from concourse import bacc, tile
from concourse.bass_utils import run_bass_kernel_spmd

FP16 = mybir.dt.float16
F32 = mybir.dt.float32

B, N, D = 8192, 16, 768
N_CORES = 8
ROWS = B * N                      # 131072
ROWS_PER_CORE = ROWS // N_CORES   # 16384
GROUPS = ROWS_PER_CORE // 128     # 128 groups of 128 rows per core
KCH = D // 128                    # 6 contraction chunks
SG = 8                            # groups per super-group (one DMA batch)
N_SG = GROUPS // SG               # 16
MASK_K = 2 + 128 // N             # 10 live mask matmul rows
SENT = np.float32(2.0 ** 14)      # fp16-exact sentinel
MARGIN = np.float32(0.3)

_CACHE: dict = {}

LAST_RESULT = None  # BassKernelResults of the most recent device run


def _build_program() -> bass.Bass:
    nc = bacc.Bacc(None)
    et = nc.declare_dram_parameter("et", [N_SG, 128, SG * D], FP16, isOutput=False)
    mrhs = nc.declare_dram_parameter("mrhs", [N_SG, MASK_K, SG * 128], FP16, isOutput=False)
    mlhs = nc.declare_dram_parameter("mlhs", [128, 128], FP16, isOutput=False)
    out = nc.declare_dram_parameter("out", [N_SG, 128, SG], F32, isOutput=True)

    with tile.TileContext(nc) as tc:
        with (
            tc.tile_pool(name="emb", bufs=3) as emb_pool,
            tc.tile_pool(name="const", bufs=1) as const_pool,
            tc.tile_pool(name="wide", bufs=2) as wide_pool,
            tc.tile_pool(name="psum", bufs=8, space="PSUM") as psum_pool,
        ):
            mlhs_t = const_pool.tile([128, 128], FP16)
            nc.scalar.dma_start(mlhs_t[:], mlhs[:])
            # Two ping-pong mask-rhs tiles; pad rows (MASK_K..127) x zero
            # lhsT rows contribute nothing -- memset once so no NaN*0.
            mr_tiles = []
            for i in range(2):
                mr_t = const_pool.tile([128, SG * 128], FP16, name=f"mr{i}")
                nc.vector.memset(mr_t[:, :], 0.0)
                mr_tiles.append(mr_t)

            HALF = SG * D // 2
            for sg in range(N_SG):
                hi = emb_pool.tile([128, SG * D], FP16, tag="hi")
                # Split each embedding load across both HWDGE rings so the
                # two halves transfer in parallel.
                nc.sync.dma_start(hi[:, :HALF], et[sg][:, :HALF])
                nc.scalar.dma_start(hi[:, HALF:], et[sg][:, HALF:])
                mr_t = mr_tiles[sg % 2]
                nc.sync.dma_start(mr_t[:MASK_K, :], mrhs[sg])
                wide = wide_pool.tile([128, SG], F32)
                for gi in range(SG):
                    ps = psum_pool.tile([128, 512], F32)  # one full PSUM bank
                    G = ps[:, 0:128]
                    # Mask sentinels first (start=True clears the bank).
                    nc.tensor.matmul(
                        G, mlhs_t[:], mr_t[:, gi * 128:(gi + 1) * 128],
                        start=True, stop=False,
                    )
                    for k in range(KCH):
                        c0 = (gi * KCH + k) * 128
                        hk = hi[:, c0:c0 + 128]
                        nc.tensor.matmul(G, hk, hk, start=False, stop=(k == KCH - 1))
                    nc.vector.reduce_max(wide[:, gi:gi + 1], G, axis=mybir.AxisListType.X)
                nc.scalar.dma_start(out[sg], wide[:])
    nc.finalize()
    return nc


def _prep_core_inputs(Xn16: np.ndarray, lab: np.ndarray):
    """Per-core input maps from fp16-normalized embeddings + flat labels."""
    negc = np.where(lab == 1, -SENT, np.float32(0.0)).astype(np.float16)

    m_idx = np.arange(128)
    # mask lhsT: [128, 128], rows = [ones; ones; u_0..u_7; zeros...]
    mlhs = np.zeros((128, 128), dtype=np.float16)
    mlhs[0, :] = 1.0
    mlhs[1, :] = 1.0
    for s in range(128 // N):
        mlhs[2 + s, :] = (m_idx // N == s).astype(np.float16)

    # static part of mask rhs rows 1..9 (per 128-column group)
    mrhs_static = np.zeros((MASK_K, 128), dtype=np.float16)
    mrhs_static[1, :] = -SENT
    for s in range(128 // N):
        mrhs_static[2 + s, :] = np.where(m_idx // N == s, SENT, 0.0).astype(np.float16)

    def to_sg_layout(a_core: np.ndarray) -> np.ndarray:
        # [rows=16384, 768] -> [sg, gi, n, k, p] -> [sg, p, gi, k, n]
        return np.ascontiguousarray(
            a_core.reshape(N_SG, SG, 128, KCH, 128).transpose(0, 4, 1, 3, 2)
        ).reshape(N_SG, 128, SG * D)

    in_maps = []
    for c in range(N_CORES):
        r0 = c * ROWS_PER_CORE
        r1 = r0 + ROWS_PER_CORE
        # mask rhs per super-group: [N_SG, 10, SG*128]
        mr = np.empty((MASK_K, ROWS_PER_CORE), dtype=np.float16)
        mr[0, :] = negc[r0:r1]
        mr[1:, :] = np.tile(mrhs_static[1:, :], (1, GROUPS))
        mr = mr.reshape(MASK_K, N_SG, SG * 128).transpose(1, 0, 2)
        in_maps.append({
            "et": to_sg_layout(Xn16[r0:r1]),
            "mrhs": np.ascontiguousarray(mr),
            "mlhs": mlhs,
        })
    return in_maps


def kernel(embeddings: np.ndarray, labels: np.ndarray) -> np.ndarray:
    global LAST_RESULT
    assert embeddings.shape == (B, N, D)
    assert labels.shape == (B, N)

    X = np.asarray(embeddings, dtype=np.float32).reshape(ROWS, D)
    lab = np.asarray(labels).reshape(ROWS)

    ss = np.square(X).sum(axis=1, dtype=np.float32)
    norms = np.sqrt(ss)
    Xn16 = (X / np.maximum(norms, np.float32(1e-12))[:, None]).astype(np.float16)

    in_maps = _prep_core_inputs(Xn16, lab)

    if "nc" not in _CACHE:
        _CACHE["nc"] = _build_program()
    nc = _CACHE["nc"]

    trace = os.environ.get("BASS_KERNEL_TRACE", "0") == "1"
    res = run_bass_kernel_spmd(nc, in_maps, list(range(N_CORES)), trace=trace)
    LAST_RESULT = res

    # out[sg, p, gi]: group g = sg*SG+gi, row-within-group p
    maxneg = np.concatenate(
        [np.asarray(r["out"]).transpose(0, 2, 1).reshape(ROWS_PER_CORE)
         for r in res.results]
    )

    triplet = np.maximum(maxneg + MARGIN, np.float32(0.0))
    has_neg = (np.asarray(labels) == 0).any(axis=1)
    w = (lab == 1) & np.repeat(has_neg, N)
    loss_sum = np.float32((triplet * w).sum(dtype=np.float64))
    count = int(w.sum())
    loss = np.float32(loss_sum / np.float32(max(count, 1)))
    return np.asarray(loss, dtype=np.float32)
